# revision 12
# baseline (speedup 1.0000x reference)
"""LSTM-encoder (VAE head) Trainium kernel, v5: replicated, zero-collective.

v4 (hidden-split across 8 cores) spent ~1.5-4 ms per step in the per-step
AllGather on this stack — 512 collectives dominated everything.  v5 removes
ALL cross-core communication: one core computes the full recurrence.

  - gates [B=128, 4H=4096] per step, banked into 8 PSUM tiles of [128,512]
    (bank-sequential so the activation chain pipelines behind the matmuls).
  - x@W: fp8 DoubleRow (2 k-tiles fused, x/4 and W*4 quantized e4m3).
  - h@U: bf16 h, bf16 U, 8 k-tiles x 512-col moving.
  - activation chain via Exp/Ln (one table set, no swaps).
  - h [B,1024] -> hT [128,8,128] via DMA block-transposes.
  - VAE head on the local full hT.

The bass program is executed through a CACHED jax.jit wrapper: the stock
run_bass_kernel_spmd rebuilds the jit on every call, which costs seconds of
re-tracing for a 45k-instruction program.  Repeat kernel() calls reuse the
traced executable and only pay input shipping + device execution.
"""

import numpy as np

import concourse.bass as bass
import concourse.mybir as mybir
import concourse.tile as tile

AF = mybir.ActivationFunctionType
F32 = mybir.dt.float32
BF16 = mybir.dt.bfloat16
FP8 = mybir.dt.float8e4

B, D, H, Z = 128, 256, 1024, 128
G4 = 4 * H                # 4096 gate cols, order [g|i|f|o]
KH = H // 128             # 8 hT k-tiles
KD = D // 128             # 2 x k-tiles
NBANK = 8                 # PSUM banks for the gate row
BW = G4 // NBANK          # 512 cols per bank
XBLK = 8                  # x prefetch block (steps per DMA)
XSCALE = 0.25             # x quantization scale for fp8 (W gets 1/XSCALE)
U_FP8 = False             # h@U in fp8 DoubleRow (h/HSCALE, U*HSCALE)
HSCALE = 8.0              # U fp8 scale (h gets 1/HSCALE; head W gets HSCALE)

_SPILL_TYPES = (
    "InstMatmult", "InstTensorTensor", "InstActivation", "InstTensorCopy",
    "InstTensorScalarPtr", "InstReciprocal", "InstMemset", "InstNoOp",
    "InstLdweights", "InstCopyPredicated", "InstTensorScalarAffineSelect",
    "InstCollectiveCompute", "InstEventSemaphore", "InstDrain", "InstDMACopy",
    "InstLoadActFuncSet", "InstTensorReduce", "InstDmaTransposeAnt",
)

_WAIT_LIMITS = {"InstDmaTransposeAnt": 0}


def _spill_excess_waits(nc, limit=1):
    f = nc.m.functions[0]
    n_spilled = 0
    for bb in f.blocks:
        out = []
        for inst in bb.instructions:
            si = inst.sync_info
            waits = list(si.on_wait) if si and si.on_wait else []
            tname = type(inst).__name__
            limit_t = _WAIT_LIMITS.get(tname, limit)
            if tname in _SPILL_TYPES and len(waits) > limit_t:
                keep = waits[len(waits) - limit_t:] if limit_t else []
                for w in waits[: len(waits) - limit_t] if limit_t else waits:
                    es = mybir.InstEventSemaphore(
                        name=f"WSPILL-{n_spilled}-{inst.name}",
                        engine=inst.engine,
                        ins=[], outs=[],
                        sync_info=mybir.SyncInfo(on_wait=[w], on_update=[]),
                    )
                    out.append(es)
                    n_spilled += 1
                si.on_wait = keep
            out.append(inst)
        bb.instructions = out
    return n_spilled


def build_nc(T: int, has_bias=False, T_data=512, spill=True):
    nc = bass.Bass("TRN2", target_bir_lowering=False, debug=False,
                   num_devices=1)

    xT_d = nc.dram_tensor("xT", [T_data, KD, 128, B], FP8, kind="ExternalInput")
    U_d = nc.dram_tensor("Usl", [KH, 128, G4], FP8 if U_FP8 else BF16,
                         kind="ExternalInput")
    W_d = nc.dram_tensor("Wsl", [KD, 128, G4], FP8, kind="ExternalInput")
    b_d = nc.dram_tensor("bsl", [1, G4], BF16, kind="ExternalInput")
    Wm_d = nc.dram_tensor("Wm", [KH, 128, Z], BF16, kind="ExternalInput")
    Wv_d = nc.dram_tensor("Wv", [KH, 128, Z], BF16, kind="ExternalInput")
    bm_d = nc.dram_tensor("bm", [Z, 1], F32, kind="ExternalInput")
    bv_d = nc.dram_tensor("bv", [Z, 1], F32, kind="ExternalInput")
    bvh_d = nc.dram_tensor("bvh", [Z, 1], F32, kind="ExternalInput")
    epsT_d = nc.dram_tensor("epsT", [Z, B], F32, kind="ExternalInput")

    muT_d = nc.dram_tensor("muT", [Z, B], F32, kind="ExternalOutput")
    lvT_d = nc.dram_tensor("logvarT", [Z, B], F32, kind="ExternalOutput")
    zT_d = nc.dram_tensor("zT", [Z, B], F32, kind="ExternalOutput")

    xblk = min(XBLK, T)
    assert T % xblk == 0 and T_data % xblk == 0

    with tile.TileContext(nc) as tc:
        with (
            tc.tile_pool(name="const", bufs=1) as cpool,
            tc.tile_pool(name="xblk", bufs=2) as x_pool,
            tc.tile_pool(name="hT", bufs=2) as hT_pool,
            tc.tile_pool(name="chain", bufs=2) as ch_pool,
            tc.tile_pool(name="gps", bufs=8, space="PSUM") as gps_pool,
        ):
            U_sb = cpool.tile([128, KH, G4], BF16, tag="U")
            nc.sync.dma_start(out=U_sb[:], in_=U_d.ap().rearrange("c p g -> p c g"))
            W_sb = cpool.tile([128, KD, G4], FP8, tag="W")
            nc.sync.dma_start(out=W_sb[:], in_=W_d.ap().rearrange("c p g -> p c g"))
            Wm_sb = cpool.tile([128, KH, Z], BF16, tag="Wm")
            nc.sync.dma_start(out=Wm_sb[:], in_=Wm_d.ap().rearrange("c p z -> p c z"))
            Wv_sb = cpool.tile([128, KH, Z], BF16, tag="Wv")
            nc.sync.dma_start(out=Wv_sb[:], in_=Wv_d.ap().rearrange("c p z -> p c z"))
            bm_sb = cpool.tile([Z, 1], F32, tag="bm")
            nc.sync.dma_start(out=bm_sb[:], in_=bm_d.ap())
            bv_sb = cpool.tile([Z, 1], F32, tag="bv")
            nc.sync.dma_start(out=bv_sb[:], in_=bv_d.ap())
            bvh_sb = cpool.tile([Z, 1], F32, tag="bvh")
            nc.sync.dma_start(out=bvh_sb[:], in_=bvh_d.ap())
            epsT_sb = cpool.tile([Z, B], F32, tag="epsT")
            nc.sync.dma_start(out=epsT_sb[:], in_=epsT_d.ap())
            if has_bias:
                b_sb = cpool.tile([1, G4], BF16, tag="b")
                nc.sync.dma_start(out=b_sb[:], in_=b_d.ap())
                ones_sb = cpool.tile([1, B], BF16, tag="ones")
                nc.vector.memset(ones_sb[:], 1.0)

            c_sb = cpool.tile([128, H], F32, tag="c")
            nc.vector.memset(c_sb[:], 0.0)
            hT_prev = None  # h(-1) = 0: step 0 emits no h-matmuls

            for t in range(T):
                if t % xblk == 0:
                    x_blk = x_pool.tile([128, xblk, KD, B], FP8, tag="xb",
                                        name=f"xb_{t}")
                    nc.sync.dma_start(
                        out=x_blk[:],
                        in_=xT_d.ap()[t:t + xblk].rearrange("t c p b -> p t c b"))
                ti = t % xblk

                # ---- gates, bank-sequential; chain ops interleave so each
                # gate block's post-processing hides under later banks'
                # matmuls.  Block layout [g|f|i|o], banks (0,1)(2,3)(4,5)(6,7):
                # g first (feeds the longest chain), f second (f*c_prev needs
                # nothing else), i third (closes c and softplus(c) while the o
                # banks' matmuls run), o last (shortest chain to h).
                # All 8 x@W matmuls are issued first (no h dependency): they
                # execute right at the step boundary while the previous step's
                # tail chain finishes, keeping the PE clock ramped.
                e_all = ch_pool.tile([128, G4], BF16, tag="eall")
                spg = ch_pool.tile([128, H], F32, tag="spg")
                sig_i = ch_pool.tile([128, H], BF16, tag="sigi")
                sig_f = ch_pool.tile([128, H], BF16, tag="sigf")
                sig_o = ch_pool.tile([128, H], BF16, tag="sigo")
                tig = ch_pool.tile([128, H], F32, tag="tig")
                tfc = ch_pool.tile([128, H], F32, tag="tfc")
                ec = ch_pool.tile([128, H], BF16, tag="ec")
                spc = ch_pool.tile([128, H], F32, tag="spc")
                h_b = ch_pool.tile([128, H], BF16, tag="h")
                # hT in two tiles so step t+1's kt0-3 matmuls depend only on
                # the first four transposes (ready one o-half earlier)
                hT_lo = hT_pool.tile([128, KH // 2, B], BF16, tag="hTlo",
                                     name=f"hTlo_{t}")
                hT_hi = hT_pool.tile([128, KH // 2, B], BF16, tag="hThi",
                                     name=f"hThi_{t}")
                last_x = hT_prev is None
                g_ps = []
                for bk in range(NBANK):
                    ps = gps_pool.tile([128, BW], F32, tag="gps",
                                       name=f"gps_{t}_{bk}")
                    g_ps.append(ps)
                    first = True
                    if has_bias:
                        nc.tensor.matmul(ps[:], ones_sb[:],
                                         b_sb[:, bk * BW:(bk + 1) * BW],
                                         start=True, stop=False)
                        first = False
                    nc.tensor.matmul(
                        ps[:],
                        x_blk[:, ti, :, :],                 # [128, KD, B] fp8
                        W_sb[:, :, bk * BW:(bk + 1) * BW],  # [128, KD, BW] fp8
                        start=first, stop=last_x,
                        perf_mode=mybir.MatmulPerfMode.DoubleRow,
                    )

                def chain_hooks(bk):
                    # (host pre-negated g cols so e_all holds
                    # [e^g|e^-f|e^-i|e^-o] after Exp(scale=-1))
                    if bk == 1:      # g complete -> softplus(g)
                        nc.scalar.activation(spg[:], e_all[:, 0:H],
                                             AF.Ln, bias=1.0)
                    elif bk in (2, 3):  # f half -> sigmoid(f), f*c_prev
                        lo = (bk - 2) * BW
                        d_f = ch_pool.tile([128, BW], BF16, tag=f"df{bk-2}")
                        nc.vector.tensor_scalar_add(
                            d_f[:], e_all[:, H + lo:H + lo + BW], 1.0)
                        with nc.allow_low_precision("bf16 sigmoid is fine"):
                            nc.vector.reciprocal(sig_f[:, lo:lo + BW], d_f[:])
                        nc.vector.tensor_mul(tfc[:, lo:lo + BW],
                                             sig_f[:, lo:lo + BW],
                                             c_sb[:, lo:lo + BW])
                    elif bk in (4, 5):  # i half -> c half + softplus(c) half
                        lo = (bk - 4) * BW
                        d_i = ch_pool.tile([128, BW], BF16, tag=f"di{bk-4}")
                        nc.vector.tensor_scalar_add(
                            d_i[:], e_all[:, 2 * H + lo:2 * H + lo + BW], 1.0)
                        with nc.allow_low_precision("bf16 sigmoid is fine"):
                            nc.vector.reciprocal(sig_i[:, lo:lo + BW], d_i[:])
                        nc.vector.tensor_mul(tig[:, lo:lo + BW],
                                             sig_i[:, lo:lo + BW],
                                             spg[:, lo:lo + BW])
                        nc.vector.tensor_add(c_sb[:, lo:lo + BW],
                                             tig[:, lo:lo + BW],
                                             tfc[:, lo:lo + BW])
                        nc.scalar.activation(ec[:, lo:lo + BW],
                                             c_sb[:, lo:lo + BW], AF.Exp)
                        nc.scalar.activation(spc[:, lo:lo + BW],
                                             ec[:, lo:lo + BW], AF.Ln,
                                             bias=1.0)
                    elif bk >= 6:    # o half complete -> h half + transposes
                        lo = (bk - 6) * BW
                        d_o = ch_pool.tile([128, BW], BF16, tag=f"do{bk-6}")
                        nc.vector.tensor_scalar_add(
                            d_o[:], e_all[:, 3 * H + lo:3 * H + lo + BW], 1.0)
                        with nc.allow_low_precision("bf16 sigmoid is fine"):
                            nc.vector.reciprocal(sig_o[:, lo:lo + BW], d_o[:])
                        with nc.allow_low_precision("h in bf16 (mm dtype)"):
                            # on gpsimd: keeps the DVE queue free for the
                            # o-sigmoid of the other half
                            nc.gpsimd.tensor_mul(h_b[:, lo:lo + BW],
                                                 sig_o[:, lo:lo + BW],
                                                 spc[:, lo:lo + BW])
                        hT_half = hT_lo if bk == 6 else hT_hi
                        for kt in range(4):
                            col = lo + kt * 128
                            nc.sync.dma_start_transpose(
                                hT_half[:, kt, :], h_b[:, col:col + 128])

                for bk in range(NBANK):
                    if not last_x:
                        for kt in range(KH):
                            src = hT_prev[0] if kt < 4 else hT_prev[1]
                            nc.tensor.matmul(
                                g_ps[bk][:], src[:, kt % 4, :],
                                U_sb[:, kt, bk * BW:(bk + 1) * BW],
                                start=False, stop=(kt == KH - 1))
                    # Exp of this bank while the next bank's matmuls run.
                    nc.scalar.activation(e_all[:, bk * BW:(bk + 1) * BW],
                                         g_ps[bk][:], AF.Exp, scale=-1.0)
                    chain_hooks(bk)
                hT_prev = (hT_lo, hT_hi)

            # ---- VAE head ----
            mu_ps = gps_pool.tile([Z, B], F32, tag="gps", name="head_mu")
            lv_ps = gps_pool.tile([Z, B], F32, tag="gps", name="head_lv")
            for c in range(KH):
                src = hT_prev[0] if c < 4 else hT_prev[1]
                nc.tensor.matmul(mu_ps[:], Wm_sb[:, c, :], src[:, c % 4, :],
                                 start=(c == 0), stop=(c == KH - 1))
            for c in range(KH):
                src = hT_prev[0] if c < 4 else hT_prev[1]
                nc.tensor.matmul(lv_ps[:], Wv_sb[:, c, :], src[:, c % 4, :],
                                 start=(c == 0), stop=(c == KH - 1))

            mu_sb = ch_pool.tile([Z, B], F32, tag="mu")
            nc.scalar.activation(mu_sb[:], mu_ps[:], AF.Identity, bias=bm_sb[:])
            lv_sb = ch_pool.tile([Z, B], F32, tag="lv")
            nc.scalar.activation(lv_sb[:], lv_ps[:], AF.Identity, bias=bv_sb[:])
            es = ch_pool.tile([Z, B], F32, tag="es")
            nc.scalar.activation(es[:], lv_ps[:], AF.Exp, bias=bvh_sb[:], scale=0.5)
            ez = ch_pool.tile([Z, B], F32, tag="ez")
            nc.vector.tensor_mul(ez[:], es[:], epsT_sb[:])
            z_sb = ch_pool.tile([Z, B], F32, tag="z")
            nc.vector.tensor_add(z_sb[:], mu_sb[:], ez[:])

            nc.sync.dma_start(out=muT_d.ap(), in_=mu_sb[:])
            nc.sync.dma_start(out=lvT_d.ap(), in_=lv_sb[:])
            nc.sync.dma_start(out=zT_d.ap(), in_=z_sb[:])

    if spill:
        _spill_excess_waits(nc)
    return nc


# ----------------------------------------------------------------------------
# Cached jit executor (run_bass_kernel_spmd rebuilds the jit every call;
# that costs seconds of retracing for this program size)
# ----------------------------------------------------------------------------

class _Exec:
    def __init__(self, nc):
        import jax
        from concourse.bass2jax import (
            _bass_exec_p, install_neuronx_cc_hook, partition_id_tensor)
        install_neuronx_cc_hook()
        self.nc = nc
        in_names, out_names, out_avals, zero_specs = [], [], [], []
        pname = nc.partition_id_tensor.name if nc.partition_id_tensor else None
        for alloc in nc.m.functions[0].allocations:
            if not isinstance(alloc, mybir.MemoryLocationSet):
                continue
            name = alloc.memorylocations[0].name
            if alloc.kind == "ExternalInput":
                if name != pname:
                    in_names.append(name)
            elif alloc.kind == "ExternalOutput":
                out_names.append(name)
                shape = tuple(alloc.tensor_shape)
                dtype = mybir.dt.np(alloc.dtype)
                out_avals.append(jax.core.ShapedArray(shape, dtype))
                zero_specs.append((shape, dtype))
        n_params = len(in_names)
        n_outs = len(out_avals)
        in_names_full = in_names + out_names + ([pname] if pname else [])

        def _body(*args):
            operands = list(args)
            if pname is not None:
                operands.append(partition_id_tensor())
            return tuple(_bass_exec_p.bind(
                *operands,
                out_avals=tuple(out_avals),
                in_names=tuple(in_names_full),
                out_names=tuple(out_names),
                lowering_input_output_aliases=(),
                sim_require_finite=True,
                sim_require_nnan=True,
                nc=nc,
            ))

        donate = tuple(range(n_params, n_params + n_outs))
        self.jf = jax.jit(_body, donate_argnums=donate, keep_unused=True)
        self.in_names = in_names
        self.out_names = out_names
        self.zero_specs = zero_specs

    def __call__(self, in_map, device_args=None):
        args = (device_args if device_args is not None
                else [np.asarray(in_map[n]) for n in self.in_names])
        zouts = [np.zeros(s, d) for s, d in self.zero_specs]
        outs = self.jf(*args, *zouts)
        return {n: np.asarray(o) for n, o in zip(self.out_names, outs)}


# ----------------------------------------------------------------------------
# Host-side packing
# ----------------------------------------------------------------------------

def make_in_maps(x, W, U, b, Wm, bm, Wv, bv, eps):
    import ml_dtypes
    bf = ml_dtypes.bfloat16
    f8 = ml_dtypes.float8_e4m3
    T = x.shape[1]

    xT = np.ascontiguousarray(x.transpose(1, 2, 0)).reshape(T, KD, 128, B)
    xT = (xT * XSCALE).astype(f8)
    epsT = np.ascontiguousarray(eps.T).astype(np.float32)
    bm_c = np.ascontiguousarray(bm.reshape(Z, 1)).astype(np.float32)
    bv_c = np.ascontiguousarray(bv.reshape(Z, 1)).astype(np.float32)
    bvh_c = np.ascontiguousarray(0.5 * bv.reshape(Z, 1)).astype(np.float32)
    Wm_r = np.ascontiguousarray(Wm.reshape(KH, 128, Z)).astype(bf)
    Wv_r = np.ascontiguousarray(Wv.reshape(KH, 128, Z)).astype(bf)

    # gate order [g | f | i | o]; Keras kernel order is i,f,g,o
    cols = np.concatenate([
        np.arange(2 * H, 3 * H),  # g
        np.arange(1 * H, 2 * H),  # f
        np.arange(0 * H, 1 * H),  # i
        np.arange(3 * H, 4 * H),  # o
    ])
    # negate the g-block columns so one Exp(scale=-1) serves all gates
    neg = np.ones((G4,), np.float32)
    neg[:H] = -1.0
    Usl = np.ascontiguousarray(U[:, cols] * neg).reshape(KH, 128, G4).astype(bf)
    Wsl = np.ascontiguousarray(W[:, cols] * neg / XSCALE).reshape(
        KD, 128, G4).astype(f8)
    bsl = (b[cols] * neg).reshape(1, G4).astype(bf)
    in_map = {
        "xT": xT, "Usl": Usl, "Wsl": Wsl, "bsl": bsl,
        "Wm": Wm_r, "Wv": Wv_r,
        "bm": bm_c, "bv": bv_c, "bvh": bvh_c, "epsT": epsT,
    }
    return [in_map]


def postprocess(core0_out):
    mu = np.ascontiguousarray(core0_out["muT"].T).astype(np.float32)
    logvar = np.ascontiguousarray(core0_out["logvarT"].T).astype(np.float32)
    z = np.ascontiguousarray(core0_out["zT"].T).astype(np.float32)
    return mu, logvar, z


_EXEC_CACHE = {}


def get_exec(T: int, has_bias=False, T_data=512):
    key = (T, has_bias, T_data)
    if key not in _EXEC_CACHE:
        _EXEC_CACHE[key] = _Exec(build_nc(T, has_bias=has_bias, T_data=T_data))
    return _EXEC_CACHE[key]


def kernel(x, W, U, b, Wm, bm, Wv, bv, eps):
    import time as _time

    x = np.asarray(x, dtype=np.float32)
    W = np.asarray(W, dtype=np.float32)
    U = np.asarray(U, dtype=np.float32)
    b = np.asarray(b, dtype=np.float32)
    Wm = np.asarray(Wm, dtype=np.float32)
    bm = np.asarray(bm, dtype=np.float32)
    Wv = np.asarray(Wv, dtype=np.float32)
    bv = np.asarray(bv, dtype=np.float32)
    eps = np.asarray(eps, dtype=np.float32)

    T = x.shape[1]
    has_bias = bool(np.any(b != 0))
    ex = get_exec(T, has_bias=has_bias, T_data=T)
    in_map = make_in_maps(x, W, U, b, Wm, bm, Wv, bv, eps)[0]
    last = None
    for _attempt in range(3):
        try:
            return postprocess(ex(in_map))
        except Exception as e:
            last = e
            _time.sleep(2.0)
    raise last


# revision 28
# speedup vs baseline: 1.2337x; 1.2337x over previous
"""LSTM-encoder (VAE head) Trainium kernel, v5: replicated, zero-collective.

v4 (hidden-split across 8 cores) spent ~1.5-4 ms per step in the per-step
AllGather on this stack — 512 collectives dominated everything.  v5 removes
ALL cross-core communication: one core computes the full recurrence.

  - gates [B=128, 4H=4096] per step, banked into 8 PSUM tiles of [128,512]
    (bank-sequential so the activation chain pipelines behind the matmuls).
  - gate block layout [g|f|i|o]: g first (feeds the longest chain), o last
    (shortest chain to h); per-block chain ops are emitted as soon as their
    banks' matmuls finish, so only the o-sigmoid tail is exposed.
  - all 8 x@W matmuls (fp8 DoubleRow, no h dependency) issue at the step
    boundary, overlapping the previous step's tail and keeping the PE ramped.
  - h@U: bf16 h, bf16 U, 8 k-tiles x 512-col moving.  (fp8-DR U measured
    36.5us/step but rel_err 3.8e-2 — over the 2e-2 gate; bf16 it stays.)
  - activation chain via Exp/Ln (one table set, no swaps); the h-muls stay
    on DVE right after the o-sigmoid reciprocal (same-engine sequencing —
    cross-engine sem hops cost ~1us on this stack; gpsimd offload measured
    slower).
  - h [B,1024] -> hT in two tiles (lo/hi k-tiles) via one 3-D DMA block
    transpose per half, so the next step's first matmuls depend only on the
    lo half and the SP queue carries 2 instructions instead of 8.
  - VAE head on the local full hT.

The bass program is executed through a CACHED jax.jit wrapper: the stock
run_bass_kernel_spmd rebuilds the jit on every call, which costs seconds of
re-tracing for a 45k-instruction program.  Repeat kernel() calls reuse the
traced executable and only pay input shipping + device execution.
"""

import numpy as np

import concourse.bass as bass
import concourse.mybir as mybir
import concourse.tile as tile

AF = mybir.ActivationFunctionType
F32 = mybir.dt.float32
BF16 = mybir.dt.bfloat16
FP8 = mybir.dt.float8e4

B, D, H, Z = 128, 256, 1024, 128
G4 = 4 * H                # 4096 gate cols, order [g|i|f|o]
KH = H // 128             # 8 hT k-tiles
KD = D // 128             # 2 x k-tiles
NBANK = 8                 # PSUM banks for the gate row
BW = G4 // NBANK          # 512 cols per bank
XBLK = 16                 # x prefetch block (steps per DMA)
XSCALE = 0.25             # x quantization scale for fp8 (W gets 1/XSCALE)
U_FP8 = False             # h@U in fp8 DoubleRow: 36.5us/step but rel_err
                          # 3.8e-2 > 2e-2 gate; bf16 U stays (1.18e-2)
HSCALE = 8.0              # U fp8 scale (h gets 1/HSCALE; head W gets HSCALE)

_SPILL_TYPES = (
    "InstMatmult", "InstTensorTensor", "InstActivation", "InstTensorCopy",
    "InstTensorScalarPtr", "InstReciprocal", "InstMemset", "InstNoOp",
    "InstLdweights", "InstCopyPredicated", "InstTensorScalarAffineSelect",
    "InstCollectiveCompute", "InstEventSemaphore", "InstDrain", "InstDMACopy",
    "InstLoadActFuncSet", "InstTensorReduce", "InstDmaTransposeAnt",
)

_WAIT_LIMITS = {"InstDmaTransposeAnt": 0}


def _spill_excess_waits(nc, limit=1):
    f = nc.m.functions[0]
    n_spilled = 0
    for bb in f.blocks:
        out = []
        for inst in bb.instructions:
            si = inst.sync_info
            waits = list(si.on_wait) if si and si.on_wait else []
            tname = type(inst).__name__
            limit_t = _WAIT_LIMITS.get(tname, limit)
            if tname in _SPILL_TYPES and len(waits) > limit_t:
                keep = waits[len(waits) - limit_t:] if limit_t else []
                for w in waits[: len(waits) - limit_t] if limit_t else waits:
                    es = mybir.InstEventSemaphore(
                        name=f"WSPILL-{n_spilled}-{inst.name}",
                        engine=inst.engine,
                        ins=[], outs=[],
                        sync_info=mybir.SyncInfo(on_wait=[w], on_update=[]),
                    )
                    out.append(es)
                    n_spilled += 1
                si.on_wait = keep
            out.append(inst)
        bb.instructions = out
    return n_spilled


def build_nc(T: int, has_bias=False, T_data=512, spill=True):
    nc = bass.Bass("TRN2", target_bir_lowering=False, debug=False,
                   num_devices=1)

    xT_d = nc.dram_tensor("xT", [T_data, KD, 128, B], FP8, kind="ExternalInput")
    U_d = nc.dram_tensor("Usl", [KH, 128, G4], FP8 if U_FP8 else BF16,
                         kind="ExternalInput")
    W_d = nc.dram_tensor("Wsl", [KD, 128, G4], FP8, kind="ExternalInput")
    b_d = nc.dram_tensor("bsl", [1, G4], BF16, kind="ExternalInput")
    Wm_d = nc.dram_tensor("Wm", [KH, 128, Z], BF16, kind="ExternalInput")
    Wv_d = nc.dram_tensor("Wv", [KH, 128, Z], BF16, kind="ExternalInput")
    bm_d = nc.dram_tensor("bm", [Z, 1], F32, kind="ExternalInput")
    bv_d = nc.dram_tensor("bv", [Z, 1], F32, kind="ExternalInput")
    bvh_d = nc.dram_tensor("bvh", [Z, 1], F32, kind="ExternalInput")
    epsT_d = nc.dram_tensor("epsT", [Z, B], F32, kind="ExternalInput")

    muT_d = nc.dram_tensor("muT", [Z, B], F32, kind="ExternalOutput")
    lvT_d = nc.dram_tensor("logvarT", [Z, B], F32, kind="ExternalOutput")
    zT_d = nc.dram_tensor("zT", [Z, B], F32, kind="ExternalOutput")

    xblk = min(XBLK, T)
    assert T % xblk == 0 and T_data % xblk == 0

    with tile.TileContext(nc) as tc:
        with (
            tc.tile_pool(name="const", bufs=1) as cpool,
            tc.tile_pool(name="xblk", bufs=2) as x_pool,
            tc.tile_pool(name="hT", bufs=2) as hT_pool,
            tc.tile_pool(name="chain", bufs=2) as ch_pool,
            tc.tile_pool(name="gps", bufs=8, space="PSUM") as gps_pool,
        ):
            U_sb = cpool.tile([128, KH, G4], FP8 if U_FP8 else BF16, tag="U")
            nc.sync.dma_start(out=U_sb[:], in_=U_d.ap().rearrange("c p g -> p c g"))
            W_sb = cpool.tile([128, KD, G4], FP8, tag="W")
            nc.sync.dma_start(out=W_sb[:], in_=W_d.ap().rearrange("c p g -> p c g"))
            Wm_sb = cpool.tile([128, KH, Z], BF16, tag="Wm")
            nc.sync.dma_start(out=Wm_sb[:], in_=Wm_d.ap().rearrange("c p z -> p c z"))
            Wv_sb = cpool.tile([128, KH, Z], BF16, tag="Wv")
            nc.sync.dma_start(out=Wv_sb[:], in_=Wv_d.ap().rearrange("c p z -> p c z"))
            bm_sb = cpool.tile([Z, 1], F32, tag="bm")
            nc.sync.dma_start(out=bm_sb[:], in_=bm_d.ap())
            bv_sb = cpool.tile([Z, 1], F32, tag="bv")
            nc.sync.dma_start(out=bv_sb[:], in_=bv_d.ap())
            bvh_sb = cpool.tile([Z, 1], F32, tag="bvh")
            nc.sync.dma_start(out=bvh_sb[:], in_=bvh_d.ap())
            epsT_sb = cpool.tile([Z, B], F32, tag="epsT")
            nc.sync.dma_start(out=epsT_sb[:], in_=epsT_d.ap())
            if has_bias:
                b_sb = cpool.tile([1, G4], BF16, tag="b")
                nc.sync.dma_start(out=b_sb[:], in_=b_d.ap())
                ones_sb = cpool.tile([1, B], BF16, tag="ones")
                nc.vector.memset(ones_sb[:], 1.0)

            c_sb = cpool.tile([128, H], F32, tag="c")
            nc.vector.memset(c_sb[:], 0.0)
            if U_FP8:
                lnhs_sb = cpool.tile([128, 1], F32, tag="lnhs")
                nc.vector.memset(lnhs_sb[:], float(np.log(HSCALE)))
            hT_prev = None  # h(-1) = 0: step 0 emits no h-matmuls

            for t in range(T):
                if t % xblk == 0:
                    x_blk = x_pool.tile([128, xblk, KD, B], FP8, tag="xb",
                                        name=f"xb_{t}")
                    nc.sync.dma_start(
                        out=x_blk[:],
                        in_=xT_d.ap()[t:t + xblk].rearrange("t c p b -> p t c b"))
                ti = t % xblk

                # ---- gates, bank-sequential; chain ops interleave so each
                # gate block's post-processing hides under later banks'
                # matmuls.  Block layout [g|f|i|o], banks (0,1)(2,3)(4,5)(6,7):
                # g first (feeds the longest chain), f second (f*c_prev needs
                # nothing else), i third (closes c and softplus(c) while the o
                # banks' matmuls run), o last (shortest chain to h).
                # All 8 x@W matmuls are issued first (no h dependency): they
                # execute right at the step boundary while the previous step's
                # tail chain finishes, keeping the PE clock ramped.
                e_all = ch_pool.tile([128, G4], BF16, tag="eall")
                spg = ch_pool.tile([128, H], F32, tag="spg")
                sig_i = ch_pool.tile([128, H], BF16, tag="sigi")
                sig_f = ch_pool.tile([128, H], BF16, tag="sigf")
                sig_o = ch_pool.tile([128, H], BF16, tag="sigo")
                tig = ch_pool.tile([128, H], F32, tag="tig")
                tfc = ch_pool.tile([128, H], F32, tag="tfc")
                ec = ch_pool.tile([128, H], BF16, tag="ec")
                spc = ch_pool.tile([128, H], F32, tag="spc")
                h_b = ch_pool.tile([128, H], BF16, tag="h")
                # hT in two tiles so step t+1's kt0-3 matmuls depend only on
                # the first four transposes (ready one o-half earlier)
                hT_lo = hT_pool.tile([128, KH // 2, B], BF16, tag="hTlo",
                                     name=f"hTlo_{t}")
                hT_hi = hT_pool.tile([128, KH // 2, B], BF16, tag="hThi",
                                     name=f"hThi_{t}")
                last_x = hT_prev is None
                g_ps = []
                for bk in range(NBANK):
                    ps = gps_pool.tile([128, BW], F32, tag="gps",
                                       name=f"gps_{t}_{bk}")
                    g_ps.append(ps)
                    first = True
                    if has_bias:
                        nc.tensor.matmul(ps[:], ones_sb[:],
                                         b_sb[:, bk * BW:(bk + 1) * BW],
                                         start=True, stop=False)
                        first = False
                    nc.tensor.matmul(
                        ps[:],
                        x_blk[:, ti, :, :],                 # [128, KD, B] fp8
                        W_sb[:, :, bk * BW:(bk + 1) * BW],  # [128, KD, BW] fp8
                        start=first, stop=last_x,
                        perf_mode=mybir.MatmulPerfMode.DoubleRow,
                    )

                def chain_hooks(bk):
                    # (host pre-negated g cols so e_all holds
                    # [e^g|e^-f|e^-i|e^-o] after Exp(scale=-1))
                    if bk == 1:      # g complete -> softplus(g)
                        nc.scalar.activation(spg[:], e_all[:, 0:H],
                                             AF.Ln, bias=1.0)
                    elif bk in (2, 3):  # f half -> sigmoid(f), f*c_prev
                        lo = (bk - 2) * BW
                        d_f = ch_pool.tile([128, BW], BF16, tag=f"df{bk-2}")
                        nc.vector.tensor_scalar_add(
                            d_f[:], e_all[:, H + lo:H + lo + BW], 1.0)
                        with nc.allow_low_precision("bf16 sigmoid is fine"):
                            nc.vector.reciprocal(sig_f[:, lo:lo + BW], d_f[:])
                        nc.vector.tensor_mul(tfc[:, lo:lo + BW],
                                             sig_f[:, lo:lo + BW],
                                             c_sb[:, lo:lo + BW])
                    elif bk in (4, 5):  # i half -> c half + softplus(c) half
                        lo = (bk - 4) * BW
                        d_i = ch_pool.tile([128, BW], BF16, tag=f"di{bk-4}")
                        nc.vector.tensor_scalar_add(
                            d_i[:], e_all[:, 2 * H + lo:2 * H + lo + BW], 1.0)
                        with nc.allow_low_precision("bf16 sigmoid is fine"):
                            nc.vector.reciprocal(sig_i[:, lo:lo + BW], d_i[:])
                        nc.vector.tensor_mul(tig[:, lo:lo + BW],
                                             sig_i[:, lo:lo + BW],
                                             spg[:, lo:lo + BW])
                        nc.vector.tensor_add(c_sb[:, lo:lo + BW],
                                             tig[:, lo:lo + BW],
                                             tfc[:, lo:lo + BW])
                        nc.scalar.activation(ec[:, lo:lo + BW],
                                             c_sb[:, lo:lo + BW], AF.Exp)
                        nc.scalar.activation(spc[:, lo:lo + BW],
                                             ec[:, lo:lo + BW], AF.Ln,
                                             bias=1.0)
                    elif bk >= 6:    # o half complete -> h half + transposes
                        # With U_FP8, h is stored scaled by 1/HSCALE (U gets
                        # HSCALE): the o-bank Exp used bias=ln(HSCALE), so
                        # e_o = HSCALE*e^-o and sigmoid/HSCALE comes free via
                        # the +HSCALE offset.
                        off = HSCALE if U_FP8 else 1.0
                        lo = (bk - 6) * BW
                        d_o = ch_pool.tile([128, BW], BF16, tag=f"do{bk-6}")
                        nc.vector.tensor_scalar_add(
                            d_o[:], e_all[:, 3 * H + lo:3 * H + lo + BW], off)
                        with nc.allow_low_precision("bf16 sigmoid is fine"):
                            nc.vector.reciprocal(sig_o[:, lo:lo + BW], d_o[:])
                        with nc.allow_low_precision("h in bf16 (mm dtype)"):
                            # on DVE right after the recip: same-engine
                            # sequencing avoids a cross-engine sem hop on the
                            # tail (measured faster than gpsimd here)
                            nc.vector.tensor_mul(h_b[:, lo:lo + BW],
                                                 sig_o[:, lo:lo + BW],
                                                 spc[:, lo:lo + BW])
                        # one 3-D block transpose per half (verified bit-exact
                        # vs four 2-D calls): [128, 4, B] <- [B, 512]
                        hT_half = hT_lo if bk == 6 else hT_hi
                        nc.sync.dma_start_transpose(hT_half[:],
                                                    h_b[:, lo:lo + BW])
                        if U_FP8:
                            hT8_half = hT8_lo if bk == 6 else hT8_hi
                            with nc.allow_low_precision("fp8 h for DR matmul"):
                                nc.vector.tensor_copy(hT8_half[:], hT_half[:])

                if U_FP8:
                    hT8_lo = hT_pool.tile([128, KH // 2, B], FP8, tag="hT8lo",
                                          name=f"hT8lo_{t}")
                    hT8_hi = hT_pool.tile([128, KH // 2, B], FP8, tag="hT8hi",
                                          name=f"hT8hi_{t}")
                for bk in range(NBANK):
                    if not last_x:
                        if U_FP8:
                            for kp in range(KH // 2):
                                src = hT8_prev[0] if kp < 2 else hT8_prev[1]
                                nc.tensor.matmul(
                                    g_ps[bk][:],
                                    src[:, 2 * (kp % 2):2 * (kp % 2) + 2, :],
                                    U_sb[:, 2 * kp:2 * kp + 2,
                                         bk * BW:(bk + 1) * BW],
                                    start=False, stop=(kp == KH // 2 - 1),
                                    perf_mode=mybir.MatmulPerfMode.DoubleRow)
                        else:
                            for kt in range(KH):
                                src = hT_prev[0] if kt < 4 else hT_prev[1]
                                nc.tensor.matmul(
                                    g_ps[bk][:], src[:, kt % 4, :],
                                    U_sb[:, kt, bk * BW:(bk + 1) * BW],
                                    start=False, stop=(kt == KH - 1))
                    # Exp of this bank while the next bank's matmuls run.
                    if U_FP8 and bk >= 6:
                        nc.scalar.activation(e_all[:, bk * BW:(bk + 1) * BW],
                                             g_ps[bk][:], AF.Exp, scale=-1.0,
                                             bias=lnhs_sb[:])
                    else:
                        nc.scalar.activation(e_all[:, bk * BW:(bk + 1) * BW],
                                             g_ps[bk][:], AF.Exp, scale=-1.0)
                    chain_hooks(bk)
                hT_prev = (hT_lo, hT_hi)
                if U_FP8:
                    hT8_prev = (hT8_lo, hT8_hi)

            # ---- VAE head ----
            mu_ps = gps_pool.tile([Z, B], F32, tag="gps", name="head_mu")
            lv_ps = gps_pool.tile([Z, B], F32, tag="gps", name="head_lv")
            for c in range(KH):
                src = hT_prev[0] if c < 4 else hT_prev[1]
                nc.tensor.matmul(mu_ps[:], Wm_sb[:, c, :], src[:, c % 4, :],
                                 start=(c == 0), stop=(c == KH - 1))
            for c in range(KH):
                src = hT_prev[0] if c < 4 else hT_prev[1]
                nc.tensor.matmul(lv_ps[:], Wv_sb[:, c, :], src[:, c % 4, :],
                                 start=(c == 0), stop=(c == KH - 1))

            mu_sb = ch_pool.tile([Z, B], F32, tag="mu")
            nc.scalar.activation(mu_sb[:], mu_ps[:], AF.Identity, bias=bm_sb[:])
            lv_sb = ch_pool.tile([Z, B], F32, tag="lv")
            nc.scalar.activation(lv_sb[:], lv_ps[:], AF.Identity, bias=bv_sb[:])
            es = ch_pool.tile([Z, B], F32, tag="es")
            nc.scalar.activation(es[:], lv_ps[:], AF.Exp, bias=bvh_sb[:], scale=0.5)
            ez = ch_pool.tile([Z, B], F32, tag="ez")
            nc.vector.tensor_mul(ez[:], es[:], epsT_sb[:])
            z_sb = ch_pool.tile([Z, B], F32, tag="z")
            nc.vector.tensor_add(z_sb[:], mu_sb[:], ez[:])

            nc.sync.dma_start(out=muT_d.ap(), in_=mu_sb[:])
            nc.sync.dma_start(out=lvT_d.ap(), in_=lv_sb[:])
            nc.sync.dma_start(out=zT_d.ap(), in_=z_sb[:])

    if spill:
        _spill_excess_waits(nc)
    return nc


# ----------------------------------------------------------------------------
# Cached jit executor (run_bass_kernel_spmd rebuilds the jit every call;
# that costs seconds of retracing for this program size)
# ----------------------------------------------------------------------------

class _Exec:
    def __init__(self, nc):
        import jax
        from concourse.bass2jax import (
            _bass_exec_p, install_neuronx_cc_hook, partition_id_tensor)
        install_neuronx_cc_hook()
        self.nc = nc
        in_names, out_names, out_avals, zero_specs = [], [], [], []
        pname = nc.partition_id_tensor.name if nc.partition_id_tensor else None
        for alloc in nc.m.functions[0].allocations:
            if not isinstance(alloc, mybir.MemoryLocationSet):
                continue
            name = alloc.memorylocations[0].name
            if alloc.kind == "ExternalInput":
                if name != pname:
                    in_names.append(name)
            elif alloc.kind == "ExternalOutput":
                out_names.append(name)
                shape = tuple(alloc.tensor_shape)
                dtype = mybir.dt.np(alloc.dtype)
                out_avals.append(jax.core.ShapedArray(shape, dtype))
                zero_specs.append((shape, dtype))
        n_params = len(in_names)
        n_outs = len(out_avals)
        in_names_full = in_names + out_names + ([pname] if pname else [])

        def _body(*args):
            operands = list(args)
            if pname is not None:
                operands.append(partition_id_tensor())
            return tuple(_bass_exec_p.bind(
                *operands,
                out_avals=tuple(out_avals),
                in_names=tuple(in_names_full),
                out_names=tuple(out_names),
                lowering_input_output_aliases=(),
                sim_require_finite=True,
                sim_require_nnan=True,
                nc=nc,
            ))

        donate = tuple(range(n_params, n_params + n_outs))
        self.jf = jax.jit(_body, donate_argnums=donate, keep_unused=True)
        self.in_names = in_names
        self.out_names = out_names
        self.zero_specs = zero_specs

    def __call__(self, in_map, device_args=None):
        args = (device_args if device_args is not None
                else [np.asarray(in_map[n]) for n in self.in_names])
        zouts = [np.zeros(s, d) for s, d in self.zero_specs]
        outs = self.jf(*args, *zouts)
        return {n: np.asarray(o) for n, o in zip(self.out_names, outs)}


# ----------------------------------------------------------------------------
# Host-side packing
# ----------------------------------------------------------------------------

def make_in_maps(x, W, U, b, Wm, bm, Wv, bv, eps):
    import ml_dtypes
    bf = ml_dtypes.bfloat16
    f8 = ml_dtypes.float8_e4m3
    T = x.shape[1]

    xT = np.ascontiguousarray(x.transpose(1, 2, 0)).reshape(T, KD, 128, B)
    xT = (xT * XSCALE).astype(f8)
    epsT = np.ascontiguousarray(eps.T).astype(np.float32)
    bm_c = np.ascontiguousarray(bm.reshape(Z, 1)).astype(np.float32)
    bv_c = np.ascontiguousarray(bv.reshape(Z, 1)).astype(np.float32)
    bvh_c = np.ascontiguousarray(0.5 * bv.reshape(Z, 1)).astype(np.float32)
    hs = HSCALE if U_FP8 else 1.0
    Wm_r = np.ascontiguousarray(Wm.reshape(KH, 128, Z) * hs).astype(bf)
    Wv_r = np.ascontiguousarray(Wv.reshape(KH, 128, Z) * hs).astype(bf)

    # gate order [g | f | i | o]; Keras kernel order is i,f,g,o
    cols = np.concatenate([
        np.arange(2 * H, 3 * H),  # g
        np.arange(1 * H, 2 * H),  # f
        np.arange(0 * H, 1 * H),  # i
        np.arange(3 * H, 4 * H),  # o
    ])
    # negate the g-block columns so one Exp(scale=-1) serves all gates
    neg = np.ones((G4,), np.float32)
    neg[:H] = -1.0
    if U_FP8:
        Usl = np.ascontiguousarray(U[:, cols] * neg * HSCALE).reshape(
            KH, 128, G4).astype(f8)
    else:
        Usl = np.ascontiguousarray(U[:, cols] * neg).reshape(
            KH, 128, G4).astype(bf)
    Wsl = np.ascontiguousarray(W[:, cols] * neg / XSCALE).reshape(
        KD, 128, G4).astype(f8)
    bsl = (b[cols] * neg).reshape(1, G4).astype(bf)
    in_map = {
        "xT": xT, "Usl": Usl, "Wsl": Wsl, "bsl": bsl,
        "Wm": Wm_r, "Wv": Wv_r,
        "bm": bm_c, "bv": bv_c, "bvh": bvh_c, "epsT": epsT,
    }
    return [in_map]


def postprocess(core0_out):
    mu = np.ascontiguousarray(core0_out["muT"].T).astype(np.float32)
    logvar = np.ascontiguousarray(core0_out["logvarT"].T).astype(np.float32)
    z = np.ascontiguousarray(core0_out["zT"].T).astype(np.float32)
    return mu, logvar, z


_EXEC_CACHE = {}


def get_exec(T: int, has_bias=False, T_data=512):
    key = (T, has_bias, T_data)
    if key not in _EXEC_CACHE:
        _EXEC_CACHE[key] = _Exec(build_nc(T, has_bias=has_bias, T_data=T_data))
    return _EXEC_CACHE[key]


def kernel(x, W, U, b, Wm, bm, Wv, bv, eps):
    import time as _time

    x = np.asarray(x, dtype=np.float32)
    W = np.asarray(W, dtype=np.float32)
    U = np.asarray(U, dtype=np.float32)
    b = np.asarray(b, dtype=np.float32)
    Wm = np.asarray(Wm, dtype=np.float32)
    bm = np.asarray(bm, dtype=np.float32)
    Wv = np.asarray(Wv, dtype=np.float32)
    bv = np.asarray(bv, dtype=np.float32)
    eps = np.asarray(eps, dtype=np.float32)

    T = x.shape[1]
    has_bias = bool(np.any(b != 0))
    ex = get_exec(T, has_bias=has_bias, T_data=T)
    in_map = make_in_maps(x, W, U, b, Wm, bm, Wv, bv, eps)[0]
    last = None
    for _attempt in range(3):
        try:
            return postprocess(ex(in_map))
        except Exception as e:
            last = e
            _time.sleep(2.0)
    raise last
